# revision 42
# baseline (speedup 1.0000x reference)
"""Bahdanau (additive) attention Trainium2 kernel.

Full-input contract: kernel(**inputs) takes the unsharded inputs
(query [16,128,256], value [16,256,256], mask [16,256], W1 [256,256],
W2 [256,256], scale [256]) and returns (context, attn_weights), both
[16,128,256] float32, matching the jax reference.

Sharding: data-parallel over batch -> 8 NeuronCores x 2 batches each.

Per-core algorithm (per batch b; t=128 query rows, s=256 kv rows, u=256):
  1. preamble: transpose query/value, then qT32[u,t] = W1^T @ query^T and
     kT16[u,s] = W2^T @ value^T with u on partitions (PE)
  2. outer-sums x[u, t, s] = q[t,u] + k[s,u]: VectorE tensor_scalar_add
     (kT16 row-block + per-partition scalar qT32[:, t]); a ~10% slice of
     rows instead uses ScalarE's fused tanh(k + q_bias) directly to
     balance the two engines
  3. tanh on ScalarE in large fused SBUF->SBUF passes, laid out
     [u, ub, t*256+s]
  4. scores[t,s] = sum_u scale_u * tanh(...): M=1 matmuls (lhsT = scale
     column) spread across the 4 PE column groups; DVE copies (fused with
     the additive mask) + small DMAs redistribute into a [t, s] tile
  5. softmax over s (no max-subtraction: |scores| <= ~13), row sums via
     activation accum_out
  6. context = attn @ value (PE, fp16 inputs, fp32 accum)
"""

import sys

if "/opt/trn_rl_repo" not in sys.path:
    sys.path.insert(0, "/opt/trn_rl_repo")

from contextlib import ExitStack

import numpy as np

import concourse.bacc as bacc
import concourse.bass as bass
import concourse.tile as tile
from concourse import mybir
from concourse.bass_utils import run_bass_kernel_spmd

F32 = mybir.dt.float32
F16 = mybir.dt.float16
U8 = mybir.dt.uint8
AF = mybir.ActivationFunctionType

N_CORES = 8
B = 2          # batches per core
T = 128        # query rows
S = 256        # kv rows
D = 256        # d_model
U = 256        # units
TC = 32        # t-rows per contraction group
TG = 16        # t-rows per outer-sum/tanh group
NDIR = 3       # t-rows per t-group computed via ScalarE fused tanh(k+q)
NEG = -30000.0


def build_bass() -> bass.Bass:
    nc = bacc.Bacc("TRN2", target_bir_lowering=False, debug=False)

    q_in = nc.dram_tensor("query", [B, T, D], F32, kind="ExternalInput")
    v_in = nc.dram_tensor("value", [B, S, D], F32, kind="ExternalInput")
    m_in = nc.dram_tensor("mask", [B, S], U8, kind="ExternalInput")
    w1_in = nc.dram_tensor("W1", [D, U], F32, kind="ExternalInput")
    w2_in = nc.dram_tensor("W2", [D, U], F32, kind="ExternalInput")
    sc_in = nc.dram_tensor("scale", [U], F32, kind="ExternalInput")
    ctx_out = nc.dram_tensor("context", [B, T, D], F32, kind="ExternalOutput")
    attn_out = nc.dram_tensor("attn", [B, T, S], F32, kind="ExternalOutput")

    id32_d = nc.inline_tensor(np.eye(128, dtype=np.float32), "id32_const")

    with tile.TileContext(nc) as tc, ExitStack() as ctx:
        singles = ctx.enter_context(tc.tile_pool(name="singles", bufs=1))
        perb = ctx.enter_context(tc.tile_pool(name="perb", bufs=2))
        ob_pool = ctx.enter_context(tc.tile_pool(name="ob", bufs=4))
        tanh_pool = ctx.enter_context(tc.tile_pool(name="tanh", bufs=2))
        p_pre = ctx.enter_context(tc.tile_pool(name="p_pre", bufs=4, space="PSUM"))
        p_scq = ctx.enter_context(tc.tile_pool(name="p_scq", bufs=3, space="PSUM"))

        # ---- constants into SBUF (large transfers split across DMA queues)
        id32 = singles.tile([128, 128], F32)
        nc.sync.dma_start(out=id32, in_=id32_d[:, :])
        w1_sb = singles.tile([128, 2, U], F32)
        nc.sync.dma_start(out=w1_sb, in_=w1_in.rearrange("(a p) u -> p a u", a=2))
        w2_sb = singles.tile([128, 2, U], F32)
        nc.sync.dma_start(out=w2_sb, in_=w2_in.rearrange("(a p) u -> p a u", a=2))
        scale_f = singles.tile([128, 2], F32)
        nc.sync.dma_start(out=scale_f, in_=sc_in.rearrange("(a p) -> p a", a=2))
        scale16 = singles.tile([128, 2], F16)
        nc.vector.tensor_copy(out=scale16, in_=scale_f)
        w1_16 = singles.tile([128, 2, U], F16)
        nc.vector.tensor_copy(out=w1_16, in_=w1_sb)
        w2_16 = singles.tile([128, 2, U], F16)
        nc.vector.tensor_copy(out=w2_16, in_=w2_sb)

        # PE warm-up during the input-DMA dead time: dependency-free junk
        # matmuls flip the HAM clock gate to 2.4 GHz before real work lands
        wjunk = singles.tile([128, 512], F16)
        nc.vector.memset(wjunk, 0.0)
        for _ in range(12):
            wp = p_scq.tile([128, 2, S], F32, tag="scq")
            nc.tensor.matmul(
                wp.rearrange("p a s -> p (a s)"),
                lhsT=wjunk[:, 0:128], rhs=wjunk,
                start=True, stop=True,
            )

        # ---------------- preambles for both batches up front, so batch 1
        # prep overlaps batch 0's main loop
        pre = []
        for b in range(B):
            query_sb = perb.tile([T, D], F32, tag="query")
            nc.sync.dma_start(out=query_sb, in_=q_in[b])
            value_sb = perb.tile([128, 2, D], F32, tag="value")
            nc.sync.dma_start(
                out=value_sb, in_=v_in[b].rearrange("(a p) d -> p a d", a=2)
            )
            mask_row = m_in[b, :]
            mask_u8 = perb.tile([T, S], U8, tag="mask_u8")
            nc.sync.dma_start(
                out=mask_u8,
                in_=bass.AP(
                    tensor=mask_row.tensor,
                    offset=mask_row.offset,
                    ap=[[0, T]] + list(mask_row.ap),
                ),
            )
            mask_f = perb.tile([T, S], F32, tag="mask_f")
            nc.vector.tensor_copy(out=mask_f, in_=mask_u8)
            maskb0 = perb.tile([T, S], F32, tag="maskb")
            # (m - 1) * 30000: 0 where mask on, -30000 where off
            nc.vector.tensor_scalar(
                out=maskb0,
                in0=mask_f,
                scalar1=-NEG,
                scalar2=NEG,
                op0=mybir.AluOpType.mult,
                op1=mybir.AluOpType.add,
            )
            # [T, 2, S] view with 0-stride middle dim for the fused stt add
            maskb = bass.AP(
                tensor=maskb0.tensor,
                offset=maskb0.offset,
                ap=[list(maskb0.ap[0]), [0, 2]] + [list(maskb0.ap[1])],
            )

            # query^T / value^T via PE transposes (cast to fp16 on copy-out)
            qT = perb.tile([128, 2, T], F16, tag="qT")
            for j in range(2):
                pt = p_pre.tile([128, 128], F32, tag="pre")
                nc.tensor.transpose(pt, query_sb[:, j * 128 : (j + 1) * 128], id32)
                nc.vector.tensor_copy(out=qT[:, j, :], in_=pt)
            vT = perb.tile([128, 2, S], F16, tag="vT")
            for sblk in range(2):
                for j in range(2):
                    pt = p_pre.tile([128, 128], F32, tag="pre")
                    nc.tensor.transpose(
                        pt, value_sb[:, sblk, j * 128 : (j + 1) * 128], id32
                    )
                    nc.vector.tensor_copy(
                        out=vT[:, j, sblk * 128 : (sblk + 1) * 128], in_=pt
                    )

            # qT32[u, t] = W1^T @ query^T ; kT16[u, s] = W2^T @ value^T
            qT32 = perb.tile([128, 2, T], F32, tag="qT32")
            for ub in range(2):
                qTp = p_pre.tile([128, T], F32, tag="pre")
                for j in range(2):
                    nc.tensor.matmul(
                        qTp,
                        lhsT=w1_16[:, j, ub * 128 : (ub + 1) * 128],
                        rhs=qT[:, j, :],
                        start=(j == 0), stop=(j == 1),
                    )
                nc.vector.tensor_copy(out=qT32[:, ub, :], in_=qTp)
            kT16 = perb.tile([128, 2, S], F16, tag="kT16")
            for ub in range(2):
                kTp = p_pre.tile([128, S], F32, tag="pre")
                for j in range(2):
                    nc.tensor.matmul(
                        kTp,
                        lhsT=w2_16[:, j, ub * 128 : (ub + 1) * 128],
                        rhs=vT[:, j, :],
                        start=(j == 0), stop=(j == 1),
                    )
                nc.vector.tensor_copy(out=kT16[:, ub, :], in_=kTp)

            v16 = perb.tile([128, 2, D], F16, tag="v16")
            nc.vector.tensor_copy(out=v16, in_=value_sb)
            pre.append(dict(maskb=maskb, qT32=qT32, kT16=kT16, v16=v16))

        # ---------------- main loops
        for b in range(B):
            maskb = pre[b]["maskb"]
            qT32 = pre[b]["qT32"]
            kT16 = pre[b]["kT16"]
            v16 = pre[b]["v16"]

            scores_sb = perb.tile([T, S], F32, tag="scores")
            exp_sb = perb.tile([T, S], F32, tag="exp")
            sums = perb.tile([T, 1], F32, tag="sums")
            inv = perb.tile([T, 1], F32, tag="inv")
            attn_f = perb.tile([T, S], F32, tag="attn_f")

            def emit_contraction(tg, grp, tanh_t):
                for qr in (2 * grp, 2 * grp + 1):
                    scq = p_scq.tile([128, 2, S], F32, tag="scq")
                    for h in range(2):
                        for i in range(4):
                            tloc = qr * 8 + h * 4 + i
                            for ub in range(2):
                                nc.tensor.matmul(
                                    scq[32 * i : 32 * i + 1, h, :],
                                    lhsT=scale16[:, ub : ub + 1],
                                    rhs=tanh_t[:, ub, tloc * S : (tloc + 1) * S],
                                    start=(ub == 0), stop=(ub == 1),
                                    tile_position=(0, 32 * i),
                                )
                    scstg = perb.tile([128, 2, S], F32, tag="scstg")
                    nc.vector.scalar_tensor_tensor(
                        out=scstg,
                        in0=scq,
                        scalar=1.0,
                        in1=maskb,
                        op0=mybir.AluOpType.mult,
                        op1=mybir.AluOpType.add,
                    )
                    for h in range(2):
                        t0 = tg * TC + qr * 8 + h * 4
                        nc.gpsimd.dma_start(
                            out=scores_sb[t0 : t0 + 4, :],
                            in_=scstg[:, h, :].rearrange(
                                "(a r) s -> a r s", r=32
                            )[:, 0, :],
                        )

            prev = None
            for tg in range(T // TC):
                tanh_t = tanh_pool.tile([128, 2, TC * S], F16, tag="tanh")
                # rows via ScalarE fused tanh(k + q_bias): balances VectorE vs
                # ScalarE load; front-loaded (first group of the kernel fully
                # direct so ScalarE has work while VectorE fills the pipeline,
                # none at the end so ScalarE isn't the straggler)
                ndir = {
                    (0, 0): TG, (0, 1): 3, (0, 2): 3, (0, 3): 2,
                    (1, 0): 2, (1, 1): 1, (1, 2): 0, (1, 3): 0,
                }[(b, tg)]
                for i in range(ndir):
                    t = tg * TC + i
                    for ub in range(2):
                        nc.scalar.activation(
                            out=tanh_t[:, ub, i * S : (i + 1) * S],
                            in_=kT16[:, ub, :],
                            func=AF.Tanh,
                            bias=qT32[:, ub, t : t + 1],
                        )
                for grp in range(TC // TG):
                    g0 = grp * TG
                    lo = max(ndir - g0, 0)
                    if lo < TG:
                        # outer sums for TG t-rows, both u-blocks (DVE fp16)
                        ob = ob_pool.tile([128, 2, TG, S], F16, tag="ob")
                        for i in range(lo, TG):
                            t = tg * TC + g0 + i
                            for ub in range(2):
                                nc.vector.tensor_scalar_add(
                                    out=ob[:, ub, i, :],
                                    in0=kT16[:, ub, :],
                                    scalar1=qT32[:, ub, t : t + 1],
                                )
                        # tanh on ScalarE; the kernel's very last pass is
                        # split in two so its latency doesn't sit fully in
                        # the batch tail
                        last = b == B - 1 and tg == T // TC - 1 and grp == 1
                        bounds = (
                            [(lo, TG // 2), (TG // 2, TG)] if last
                            else [(lo, TG)]
                        )
                        for (i0, i1) in bounds:
                            dst = tanh_t[
                                :, :, (g0 + i0) * S : (g0 + i1) * S
                            ].rearrange("p a (i s) -> p a i s", s=S)
                            nc.scalar.activation(
                                out=dst, in_=ob[:, :, i0:i1, :], func=AF.Tanh
                            )
                    # contraction for the PREVIOUS tanh pass: PE contracts
                    # group g while ScalarE computes group g+1
                    if prev is not None:
                        emit_contraction(*prev)
                    prev = (tg, grp, tanh_t)
            emit_contraction(*prev)

            # ---------------- softmax over s
            nc.scalar.activation(
                out=exp_sb, in_=scores_sb, func=AF.Exp, accum_out=sums
            )
            nc.vector.reciprocal(out=inv, in_=sums)
            nc.vector.tensor_scalar_mul(out=attn_f, in0=exp_sb, scalar1=inv)
            for h in range(2):
                nc.sync.dma_start(
                    out=attn_out[b, :, h * 128 : (h + 1) * 128],
                    in_=attn_f[:, h * 128 : (h + 1) * 128],
                )

            # ---------------- context = attn @ value
            attnT = perb.tile([128, 2, T], F16, tag="attnT")
            for sblk in range(2):
                pt = p_pre.tile([128, 128], F32, tag="pre")
                nc.tensor.transpose(pt, exp_sb[:, sblk * 128 : (sblk + 1) * 128], id32)
                nc.vector.tensor_copy(out=attnT[:, sblk, :], in_=pt)
            ctxp = p_pre.tile([T, D], F32, tag="pre")
            for sblk in range(2):
                nc.tensor.matmul(
                    ctxp,
                    lhsT=attnT[:, sblk, :],
                    rhs=v16[:, sblk, :],
                    start=(sblk == 0), stop=(sblk == 1),
                )
            ctx_f = perb.tile([T, D], F32, tag="ctx_f")
            nc.vector.tensor_scalar_mul(out=ctx_f, in0=ctxp, scalar1=inv)
            for h in range(2):
                nc.sync.dma_start(
                    out=ctx_out[b, :, h * 128 : (h + 1) * 128],
                    in_=ctx_f[:, h * 128 : (h + 1) * 128],
                )

    nc.compile()
    return nc


_BUILT: bass.Bass | None = None


def _get_built() -> bass.Bass:
    global _BUILT
    if _BUILT is None:
        _BUILT = build_bass()
    return _BUILT


def make_in_maps(query, value, mask, W1, W2, scale):
    q = np.ascontiguousarray(np.asarray(query, dtype=np.float32))
    v = np.ascontiguousarray(np.asarray(value, dtype=np.float32))
    m = np.ascontiguousarray(np.asarray(mask).astype(np.uint8))
    w1 = np.ascontiguousarray(np.asarray(W1, dtype=np.float32))
    w2 = np.ascontiguousarray(np.asarray(W2, dtype=np.float32))
    sc = np.ascontiguousarray(np.asarray(scale, dtype=np.float32))
    in_maps = []
    for c in range(N_CORES):
        sl = slice(B * c, B * (c + 1))
        in_maps.append(
            {
                "query": np.ascontiguousarray(q[sl]),
                "value": np.ascontiguousarray(v[sl]),
                "mask": np.ascontiguousarray(m[sl]),
                "W1": w1,
                "W2": w2,
                "scale": sc,
            }
        )
    return in_maps


def run(query, value, mask, W1, W2, scale, trace=False, **trace_kwargs):
    nc = _get_built()
    in_maps = make_in_maps(query, value, mask, W1, W2, scale)
    res = run_bass_kernel_spmd(
        nc, in_maps, core_ids=list(range(N_CORES)), trace=trace, **trace_kwargs
    )
    context = np.concatenate([r["context"] for r in res.results], axis=0)
    attn = np.concatenate([r["attn"] for r in res.results], axis=0)
    return (context, attn), res


def kernel(query, value, mask, W1, W2, scale):
    (context, attn), _ = run(query, value, mask, W1, W2, scale, trace=False)
    return context, attn


if __name__ == "__main__":
    build_bass()
    print("build OK")


# revision 43
# speedup vs baseline: 1.0049x; 1.0049x over previous
"""Bahdanau (additive) attention Trainium2 kernel.

Full-input contract: kernel(**inputs) takes the unsharded inputs
(query [16,128,256], value [16,256,256], mask [16,256], W1 [256,256],
W2 [256,256], scale [256]) and returns (context, attn_weights), both
[16,128,256] float32, matching the jax reference.

Sharding: data-parallel over batch -> 8 NeuronCores x 2 batches each.

Per-core algorithm (per batch b; t=128 query rows, s=256 kv rows, u=256):
  1. preamble: transpose query/value, then qT32[u,t] = W1^T @ query^T and
     kT16[u,s] = W2^T @ value^T with u on partitions (PE)
  2. outer-sums x[u, t, s] = q[t,u] + k[s,u]: VectorE tensor_scalar_add
     (kT16 row-block + per-partition scalar qT32[:, t]); a ~10% slice of
     rows instead uses ScalarE's fused tanh(k + q_bias) directly to
     balance the two engines
  3. tanh on ScalarE in large fused SBUF->SBUF passes, laid out
     [u, ub, t*256+s]
  4. scores[t,s] = sum_u scale_u * tanh(...): M=1 matmuls (lhsT = scale
     column) spread across the 4 PE column groups; DVE copies (fused with
     the additive mask) + small DMAs redistribute into a [t, s] tile
  5. softmax over s (no max-subtraction: |scores| <= ~13), row sums via
     activation accum_out
  6. context = attn @ value (PE, fp16 inputs, fp32 accum)
"""

import sys

if "/opt/trn_rl_repo" not in sys.path:
    sys.path.insert(0, "/opt/trn_rl_repo")

from contextlib import ExitStack

import numpy as np

import concourse.bacc as bacc
import concourse.bass as bass
import concourse.tile as tile
from concourse import mybir
from concourse.bass_utils import run_bass_kernel_spmd

F32 = mybir.dt.float32
F16 = mybir.dt.float16
U8 = mybir.dt.uint8
AF = mybir.ActivationFunctionType

N_CORES = 8
B = 2          # batches per core
T = 128        # query rows
S = 256        # kv rows
D = 256        # d_model
U = 256        # units
TC = 32        # t-rows per contraction group
TG = 16        # t-rows per outer-sum/tanh group
NDIR = 3       # t-rows per t-group computed via ScalarE fused tanh(k+q)
NEG = -30000.0


def build_bass() -> bass.Bass:
    nc = bacc.Bacc("TRN2", target_bir_lowering=False, debug=False)

    q_in = nc.dram_tensor("query", [B, T, D], F32, kind="ExternalInput")
    v_in = nc.dram_tensor("value", [B, S, D], F32, kind="ExternalInput")
    m_in = nc.dram_tensor("mask", [B, S], U8, kind="ExternalInput")
    w1_in = nc.dram_tensor("W1", [D, U], F32, kind="ExternalInput")
    w2_in = nc.dram_tensor("W2", [D, U], F32, kind="ExternalInput")
    sc_in = nc.dram_tensor("scale", [U], F32, kind="ExternalInput")
    ctx_out = nc.dram_tensor("context", [B, T, D], F32, kind="ExternalOutput")
    attn_out = nc.dram_tensor("attn", [B, T, S], F32, kind="ExternalOutput")

    id32_d = nc.inline_tensor(np.eye(128, dtype=np.float32), "id32_const")

    with tile.TileContext(nc) as tc, ExitStack() as ctx:
        singles = ctx.enter_context(tc.tile_pool(name="singles", bufs=1))
        perb = ctx.enter_context(tc.tile_pool(name="perb", bufs=2))
        ob_pool = ctx.enter_context(tc.tile_pool(name="ob", bufs=4))
        tanh_pool = ctx.enter_context(tc.tile_pool(name="tanh", bufs=2))
        p_pre = ctx.enter_context(tc.tile_pool(name="p_pre", bufs=4, space="PSUM"))
        p_scq = ctx.enter_context(tc.tile_pool(name="p_scq", bufs=3, space="PSUM"))

        # ---- constants into SBUF (large transfers split across DMA queues)
        id32 = singles.tile([128, 128], F32)
        nc.sync.dma_start(out=id32, in_=id32_d[:, :])
        w1_sb = singles.tile([128, 2, U], F32)
        nc.sync.dma_start(out=w1_sb, in_=w1_in.rearrange("(a p) u -> p a u", a=2))
        w2_sb = singles.tile([128, 2, U], F32)
        nc.sync.dma_start(out=w2_sb, in_=w2_in.rearrange("(a p) u -> p a u", a=2))
        scale_f = singles.tile([128, 2], F32)
        nc.sync.dma_start(out=scale_f, in_=sc_in.rearrange("(a p) -> p a", a=2))
        scale16 = singles.tile([128, 2], F16)
        nc.vector.tensor_copy(out=scale16, in_=scale_f)
        w1_16 = singles.tile([128, 2, U], F16)
        nc.vector.tensor_copy(out=w1_16, in_=w1_sb)
        w2_16 = singles.tile([128, 2, U], F16)
        nc.vector.tensor_copy(out=w2_16, in_=w2_sb)

        # PE warm-up during the input-DMA dead time: dependency-free junk
        # matmuls flip the HAM clock gate to 2.4 GHz before real work lands
        wjunk = singles.tile([128, 512], F16)
        nc.vector.memset(wjunk, 0.0)
        for _ in range(12):
            wp = p_scq.tile([128, 2, S], F32, tag="scq")
            nc.tensor.matmul(
                wp.rearrange("p a s -> p (a s)"),
                lhsT=wjunk[:, 0:128], rhs=wjunk,
                start=True, stop=True,
            )

        # ---------------- preambles for both batches up front, so batch 1
        # prep overlaps batch 0's main loop
        pre = []
        for b in range(B):
            query_sb = perb.tile([T, D], F32, tag="query")
            nc.sync.dma_start(out=query_sb, in_=q_in[b])
            value_sb = perb.tile([128, 2, D], F32, tag="value")
            nc.sync.dma_start(
                out=value_sb, in_=v_in[b].rearrange("(a p) d -> p a d", a=2)
            )
            mask_row = m_in[b, :]
            mask_u8 = perb.tile([T, S], U8, tag="mask_u8")
            nc.sync.dma_start(
                out=mask_u8,
                in_=bass.AP(
                    tensor=mask_row.tensor,
                    offset=mask_row.offset,
                    ap=[[0, T]] + list(mask_row.ap),
                ),
            )
            mask_f = perb.tile([T, S], F32, tag="mask_f")
            nc.vector.tensor_copy(out=mask_f, in_=mask_u8)
            maskb0 = perb.tile([T, S], F32, tag="maskb")
            # (m - 1) * 30000: 0 where mask on, -30000 where off
            nc.vector.tensor_scalar(
                out=maskb0,
                in0=mask_f,
                scalar1=-NEG,
                scalar2=NEG,
                op0=mybir.AluOpType.mult,
                op1=mybir.AluOpType.add,
            )
            # [T, 2, S] view with 0-stride middle dim for the fused stt add
            maskb = bass.AP(
                tensor=maskb0.tensor,
                offset=maskb0.offset,
                ap=[list(maskb0.ap[0]), [0, 2]] + [list(maskb0.ap[1])],
            )

            # query^T / value^T via PE transposes (cast to fp16 on copy-out)
            qT = perb.tile([128, 2, T], F16, tag="qT")
            for j in range(2):
                pt = p_pre.tile([128, 128], F32, tag="pre")
                nc.tensor.transpose(pt, query_sb[:, j * 128 : (j + 1) * 128], id32)
                nc.vector.tensor_copy(out=qT[:, j, :], in_=pt)
            vT = perb.tile([128, 2, S], F16, tag="vT")
            for sblk in range(2):
                for j in range(2):
                    pt = p_pre.tile([128, 128], F32, tag="pre")
                    nc.tensor.transpose(
                        pt, value_sb[:, sblk, j * 128 : (j + 1) * 128], id32
                    )
                    nc.vector.tensor_copy(
                        out=vT[:, j, sblk * 128 : (sblk + 1) * 128], in_=pt
                    )

            # qT32[u, t] = W1^T @ query^T ; kT16[u, s] = W2^T @ value^T
            qT32 = perb.tile([128, 2, T], F32, tag="qT32")
            for ub in range(2):
                qTp = p_pre.tile([128, T], F32, tag="pre")
                for j in range(2):
                    nc.tensor.matmul(
                        qTp,
                        lhsT=w1_16[:, j, ub * 128 : (ub + 1) * 128],
                        rhs=qT[:, j, :],
                        start=(j == 0), stop=(j == 1),
                    )
                nc.vector.tensor_copy(out=qT32[:, ub, :], in_=qTp)
            kT16 = perb.tile([128, 2, S], F16, tag="kT16")
            for ub in range(2):
                kTp = p_pre.tile([128, S], F32, tag="pre")
                for j in range(2):
                    nc.tensor.matmul(
                        kTp,
                        lhsT=w2_16[:, j, ub * 128 : (ub + 1) * 128],
                        rhs=vT[:, j, :],
                        start=(j == 0), stop=(j == 1),
                    )
                nc.vector.tensor_copy(out=kT16[:, ub, :], in_=kTp)

            v16 = perb.tile([128, 2, D], F16, tag="v16")
            nc.vector.tensor_copy(out=v16, in_=value_sb)
            pre.append(dict(maskb=maskb, qT32=qT32, kT16=kT16, v16=v16))

        # ---------------- main loops
        for b in range(B):
            maskb = pre[b]["maskb"]
            qT32 = pre[b]["qT32"]
            kT16 = pre[b]["kT16"]
            v16 = pre[b]["v16"]

            scores_sb = perb.tile([T, S], F32, tag="scores")
            exp_sb = perb.tile([T, S], F32, tag="exp")
            sums = perb.tile([T, 1], F32, tag="sums")
            inv = perb.tile([T, 1], F32, tag="inv")
            attn_f = perb.tile([T, S], F32, tag="attn_f")

            def emit_contraction(tg, grp, tanh_t):
                for qr in (2 * grp, 2 * grp + 1):
                    scq = p_scq.tile([128, 2, S], F32, tag="scq")
                    for h in range(2):
                        for i in range(4):
                            tloc = qr * 8 + h * 4 + i
                            for ub in range(2):
                                nc.tensor.matmul(
                                    scq[32 * i : 32 * i + 1, h, :],
                                    lhsT=scale16[:, ub : ub + 1],
                                    rhs=tanh_t[:, ub, tloc * S : (tloc + 1) * S],
                                    start=(ub == 0), stop=(ub == 1),
                                    tile_position=(0, 32 * i),
                                )
                    scstg = perb.tile([128, 2, S], F32, tag="scstg")
                    nc.vector.scalar_tensor_tensor(
                        out=scstg,
                        in0=scq,
                        scalar=1.0,
                        in1=maskb,
                        op0=mybir.AluOpType.mult,
                        op1=mybir.AluOpType.add,
                    )
                    for h in range(2):
                        t0 = tg * TC + qr * 8 + h * 4
                        nc.gpsimd.dma_start(
                            out=scores_sb[t0 : t0 + 4, :],
                            in_=scstg[:, h, :].rearrange(
                                "(a r) s -> a r s", r=32
                            )[:, 0, :],
                        )

            prev = None
            for tg in range(T // TC):
                tanh_t = tanh_pool.tile([128, 2, TC * S], F16, tag="tanh")
                # rows via ScalarE fused tanh(k + q_bias): balances VectorE vs
                # ScalarE load; front-loaded (first group of the kernel fully
                # direct so ScalarE has work while VectorE fills the pipeline,
                # none at the end so ScalarE isn't the straggler)
                ndir = {
                    (0, 0): TG, (0, 1): 3, (0, 2): 3, (0, 3): 2,
                    (1, 0): 2, (1, 1): 1, (1, 2): 0, (1, 3): 0,
                }[(b, tg)]
                for i in range(ndir):
                    t = tg * TC + i
                    for ub in range(2):
                        nc.scalar.activation(
                            out=tanh_t[:, ub, i * S : (i + 1) * S],
                            in_=kT16[:, ub, :],
                            func=AF.Tanh,
                            bias=qT32[:, ub, t : t + 1],
                        )
                for grp in range(TC // TG):
                    g0 = grp * TG
                    lo = max(ndir - g0, 0)
                    if lo < TG:
                        # outer sums for TG t-rows, both u-blocks (DVE fp16)
                        ob = ob_pool.tile([128, 2, TG, S], F16, tag="ob")
                        for i in range(lo, TG):
                            t = tg * TC + g0 + i
                            for ub in range(2):
                                nc.vector.tensor_scalar_add(
                                    out=ob[:, ub, i, :],
                                    in0=kT16[:, ub, :],
                                    scalar1=qT32[:, ub, t : t + 1],
                                )
                        # tanh on ScalarE; the kernel's very last pass is
                        # split in two so its latency doesn't sit fully in
                        # the batch tail
                        last = b == B - 1 and tg == T // TC - 1 and grp == 1
                        bounds = (
                            [(lo, TG // 2), (TG // 2, TG)] if last
                            else [(lo, TG)]
                        )
                        for (i0, i1) in bounds:
                            dst = tanh_t[
                                :, :, (g0 + i0) * S : (g0 + i1) * S
                            ].rearrange("p a (i s) -> p a i s", s=S)
                            nc.scalar.activation(
                                out=dst, in_=ob[:, :, i0:i1, :], func=AF.Tanh
                            )
                    # contraction for the PREVIOUS tanh pass: PE contracts
                    # group g while ScalarE computes group g+1
                    if prev is not None:
                        emit_contraction(*prev)
                    prev = (tg, grp, tanh_t)
            emit_contraction(*prev)

            # ---------------- softmax over s
            nc.scalar.activation(
                out=exp_sb, in_=scores_sb, func=AF.Exp, accum_out=sums
            )
            nc.vector.reciprocal(out=inv, in_=sums)
            nc.vector.tensor_scalar_mul(out=attn_f, in0=exp_sb, scalar1=inv)
            nc.sync.dma_start(out=attn_out[b], in_=attn_f)

            # ---------------- context = attn @ value
            attnT = perb.tile([128, 2, T], F16, tag="attnT")
            for sblk in range(2):
                pt = p_pre.tile([128, 128], F32, tag="pre")
                nc.tensor.transpose(pt, exp_sb[:, sblk * 128 : (sblk + 1) * 128], id32)
                nc.vector.tensor_copy(out=attnT[:, sblk, :], in_=pt)
            ctxp = p_pre.tile([T, D], F32, tag="pre")
            for sblk in range(2):
                nc.tensor.matmul(
                    ctxp,
                    lhsT=attnT[:, sblk, :],
                    rhs=v16[:, sblk, :],
                    start=(sblk == 0), stop=(sblk == 1),
                )
            ctx_f = perb.tile([T, D], F32, tag="ctx_f")
            nc.vector.tensor_scalar_mul(out=ctx_f, in0=ctxp, scalar1=inv)
            nc.sync.dma_start(out=ctx_out[b], in_=ctx_f)

    nc.compile()
    return nc


_BUILT: bass.Bass | None = None


def _get_built() -> bass.Bass:
    global _BUILT
    if _BUILT is None:
        _BUILT = build_bass()
    return _BUILT


def make_in_maps(query, value, mask, W1, W2, scale):
    q = np.ascontiguousarray(np.asarray(query, dtype=np.float32))
    v = np.ascontiguousarray(np.asarray(value, dtype=np.float32))
    m = np.ascontiguousarray(np.asarray(mask).astype(np.uint8))
    w1 = np.ascontiguousarray(np.asarray(W1, dtype=np.float32))
    w2 = np.ascontiguousarray(np.asarray(W2, dtype=np.float32))
    sc = np.ascontiguousarray(np.asarray(scale, dtype=np.float32))
    in_maps = []
    for c in range(N_CORES):
        sl = slice(B * c, B * (c + 1))
        in_maps.append(
            {
                "query": np.ascontiguousarray(q[sl]),
                "value": np.ascontiguousarray(v[sl]),
                "mask": np.ascontiguousarray(m[sl]),
                "W1": w1,
                "W2": w2,
                "scale": sc,
            }
        )
    return in_maps


def run(query, value, mask, W1, W2, scale, trace=False, **trace_kwargs):
    nc = _get_built()
    in_maps = make_in_maps(query, value, mask, W1, W2, scale)
    res = run_bass_kernel_spmd(
        nc, in_maps, core_ids=list(range(N_CORES)), trace=trace, **trace_kwargs
    )
    context = np.concatenate([r["context"] for r in res.results], axis=0)
    attn = np.concatenate([r["attn"] for r in res.results], axis=0)
    return (context, attn), res


def kernel(query, value, mask, W1, W2, scale):
    (context, attn), _ = run(query, value, mask, W1, W2, scale, trace=False)
    return context, attn


if __name__ == "__main__":
    build_bass()
    print("build OK")


# revision 44
# speedup vs baseline: 1.0114x; 1.0065x over previous
"""Bahdanau (additive) attention Trainium2 kernel.

Full-input contract: kernel(**inputs) takes the unsharded inputs
(query [16,128,256], value [16,256,256], mask [16,256], W1 [256,256],
W2 [256,256], scale [256]) and returns (context, attn_weights), both
[16,128,256] float32, matching the jax reference.

Sharding: data-parallel over batch -> 8 NeuronCores x 2 batches each.

Per-core algorithm (per batch b; t=128 query rows, s=256 kv rows, u=256):
  1. preamble: transpose query/value, then qT32[u,t] = W1^T @ query^T and
     kT16[u,s] = W2^T @ value^T with u on partitions (PE)
  2. outer-sums x[u, t, s] = q[t,u] + k[s,u]: VectorE tensor_scalar_add
     (kT16 row-block + per-partition scalar qT32[:, t]); a ~10% slice of
     rows instead uses ScalarE's fused tanh(k + q_bias) directly to
     balance the two engines
  3. tanh on ScalarE in large fused SBUF->SBUF passes, laid out
     [u, ub, t*256+s]
  4. scores[t,s] = sum_u scale_u * tanh(...): M=1 matmuls (lhsT = scale
     column) spread across the 4 PE column groups; DVE copies (fused with
     the additive mask) + small DMAs redistribute into a [t, s] tile
  5. softmax over s (no max-subtraction: |scores| <= ~13), row sums via
     activation accum_out
  6. context = attn @ value (PE, fp16 inputs, fp32 accum)
"""

import sys

if "/opt/trn_rl_repo" not in sys.path:
    sys.path.insert(0, "/opt/trn_rl_repo")

from contextlib import ExitStack

import numpy as np

import concourse.bacc as bacc
import concourse.bass as bass
import concourse.tile as tile
from concourse import mybir
from concourse.bass_utils import run_bass_kernel_spmd

F32 = mybir.dt.float32
F16 = mybir.dt.float16
U8 = mybir.dt.uint8
AF = mybir.ActivationFunctionType

N_CORES = 8
B = 2          # batches per core
T = 128        # query rows
S = 256        # kv rows
D = 256        # d_model
U = 256        # units
TC = 32        # t-rows per contraction group
TG = 16        # t-rows per outer-sum/tanh group
NDIR = 3       # t-rows per t-group computed via ScalarE fused tanh(k+q)
NEG = -30000.0


def build_bass() -> bass.Bass:
    nc = bacc.Bacc("TRN2", target_bir_lowering=False, debug=False)

    q_in = nc.dram_tensor("query", [B, T, D], F32, kind="ExternalInput")
    v_in = nc.dram_tensor("value", [B, S, D], F32, kind="ExternalInput")
    m_in = nc.dram_tensor("mask", [B, S], U8, kind="ExternalInput")
    w1_in = nc.dram_tensor("W1", [D, U], F32, kind="ExternalInput")
    w2_in = nc.dram_tensor("W2", [D, U], F32, kind="ExternalInput")
    sc_in = nc.dram_tensor("scale", [U], F32, kind="ExternalInput")
    ctx_out = nc.dram_tensor("context", [B, T, D], F32, kind="ExternalOutput")
    attn_out = nc.dram_tensor("attn", [B, T, S], F32, kind="ExternalOutput")

    id32_d = nc.inline_tensor(np.eye(128, dtype=np.float32), "id32_const")

    with tile.TileContext(nc) as tc, ExitStack() as ctx:
        singles = ctx.enter_context(tc.tile_pool(name="singles", bufs=1))
        perb = ctx.enter_context(tc.tile_pool(name="perb", bufs=2))
        ob_pool = ctx.enter_context(tc.tile_pool(name="ob", bufs=4))
        tanh_pool = ctx.enter_context(tc.tile_pool(name="tanh", bufs=2))
        p_pre = ctx.enter_context(tc.tile_pool(name="p_pre", bufs=4, space="PSUM"))
        p_scq = ctx.enter_context(tc.tile_pool(name="p_scq", bufs=3, space="PSUM"))

        # ---- constants into SBUF
        id32 = singles.tile([128, 128], F32)
        nc.sync.dma_start(out=id32, in_=id32_d[:, :])
        w1_sb = singles.tile([128, 2, U], F32)
        nc.sync.dma_start(out=w1_sb, in_=w1_in.rearrange("(a p) u -> p a u", a=2))
        w2_sb = singles.tile([128, 2, U], F32)
        nc.sync.dma_start(out=w2_sb, in_=w2_in.rearrange("(a p) u -> p a u", a=2))
        scale_f = singles.tile([128, 2], F32)
        nc.sync.dma_start(out=scale_f, in_=sc_in.rearrange("(a p) -> p a", a=2))
        scale16 = singles.tile([128, 2], F16)
        nc.vector.tensor_copy(out=scale16, in_=scale_f)
        w1_16 = singles.tile([128, 2, U], F16)
        nc.vector.tensor_copy(out=w1_16, in_=w1_sb)
        w2_16 = singles.tile([128, 2, U], F16)
        nc.vector.tensor_copy(out=w2_16, in_=w2_sb)

        # PE warm-up during the input-DMA dead time: dependency-free junk
        # matmuls flip the HAM clock gate to 2.4 GHz before real work lands
        wjunk = singles.tile([128, 512], F16)
        nc.vector.memset(wjunk, 0.0)
        for _ in range(12):
            wp = p_scq.tile([128, 2, S], F32, tag="scq")
            nc.tensor.matmul(
                wp.rearrange("p a s -> p (a s)"),
                lhsT=wjunk[:, 0:128], rhs=wjunk,
                start=True, stop=True,
            )

        # ---------------- preambles for both batches up front, so batch 1
        # prep overlaps batch 0's main loop
        pre = []
        for b in range(B):
            query_sb = perb.tile([T, D], F32, tag="query")
            nc.sync.dma_start(out=query_sb, in_=q_in[b])
            value_sb = perb.tile([128, 2, D], F32, tag="value")
            nc.sync.dma_start(
                out=value_sb, in_=v_in[b].rearrange("(a p) d -> p a d", a=2)
            )
            mask_row = m_in[b, :]
            mask_u8 = perb.tile([T, S], U8, tag="mask_u8")
            nc.sync.dma_start(
                out=mask_u8,
                in_=bass.AP(
                    tensor=mask_row.tensor,
                    offset=mask_row.offset,
                    ap=[[0, T]] + list(mask_row.ap),
                ),
            )
            mask_f = perb.tile([T, S], F32, tag="mask_f")
            nc.vector.tensor_copy(out=mask_f, in_=mask_u8)
            maskb0 = perb.tile([T, S], F32, tag="maskb")
            # (m - 1) * 30000: 0 where mask on, -30000 where off
            nc.vector.tensor_scalar(
                out=maskb0,
                in0=mask_f,
                scalar1=-NEG,
                scalar2=NEG,
                op0=mybir.AluOpType.mult,
                op1=mybir.AluOpType.add,
            )
            # [T, 2, S] view with 0-stride middle dim for the fused stt add
            maskb = bass.AP(
                tensor=maskb0.tensor,
                offset=maskb0.offset,
                ap=[list(maskb0.ap[0]), [0, 2]] + [list(maskb0.ap[1])],
            )

            # query^T / value^T via PE transposes (cast to fp16 on copy-out)
            qT = perb.tile([128, 2, T], F16, tag="qT")
            for j in range(2):
                pt = p_pre.tile([128, 128], F32, tag="pre")
                nc.tensor.transpose(pt, query_sb[:, j * 128 : (j + 1) * 128], id32)
                nc.vector.tensor_copy(out=qT[:, j, :], in_=pt)
            vT = perb.tile([128, 2, S], F16, tag="vT")
            for sblk in range(2):
                for j in range(2):
                    pt = p_pre.tile([128, 128], F32, tag="pre")
                    nc.tensor.transpose(
                        pt, value_sb[:, sblk, j * 128 : (j + 1) * 128], id32
                    )
                    nc.vector.tensor_copy(
                        out=vT[:, j, sblk * 128 : (sblk + 1) * 128], in_=pt
                    )

            # qT32[u, t] = W1^T @ query^T ; kT16[u, s] = W2^T @ value^T
            qT32 = perb.tile([128, 2, T], F32, tag="qT32")
            for ub in range(2):
                qTp = p_pre.tile([128, T], F32, tag="pre")
                for j in range(2):
                    nc.tensor.matmul(
                        qTp,
                        lhsT=w1_16[:, j, ub * 128 : (ub + 1) * 128],
                        rhs=qT[:, j, :],
                        start=(j == 0), stop=(j == 1),
                    )
                nc.vector.tensor_copy(out=qT32[:, ub, :], in_=qTp)
            kT16 = perb.tile([128, 2, S], F16, tag="kT16")
            for ub in range(2):
                kTp = p_pre.tile([128, S], F32, tag="pre")
                for j in range(2):
                    nc.tensor.matmul(
                        kTp,
                        lhsT=w2_16[:, j, ub * 128 : (ub + 1) * 128],
                        rhs=vT[:, j, :],
                        start=(j == 0), stop=(j == 1),
                    )
                nc.vector.tensor_copy(out=kT16[:, ub, :], in_=kTp)

            v16 = perb.tile([128, 2, D], F16, tag="v16")
            nc.vector.tensor_copy(out=v16, in_=value_sb)
            pre.append(dict(maskb=maskb, qT32=qT32, kT16=kT16, v16=v16))

        # ---------------- main loops
        for b in range(B):
            maskb = pre[b]["maskb"]
            qT32 = pre[b]["qT32"]
            kT16 = pre[b]["kT16"]
            v16 = pre[b]["v16"]

            scores_sb = perb.tile([T, S], F32, tag="scores")
            exp_sb = perb.tile([T, S], F32, tag="exp")
            sums = perb.tile([T, 1], F32, tag="sums")
            inv = perb.tile([T, 1], F32, tag="inv")
            attn_f = perb.tile([T, S], F32, tag="attn_f")

            def emit_contraction(tg, grp, tanh_t):
                for qr in (2 * grp, 2 * grp + 1):
                    scq = p_scq.tile([128, 2, S], F32, tag="scq")
                    for h in range(2):
                        for i in range(4):
                            tloc = qr * 8 + h * 4 + i
                            for ub in range(2):
                                nc.tensor.matmul(
                                    scq[32 * i : 32 * i + 1, h, :],
                                    lhsT=scale16[:, ub : ub + 1],
                                    rhs=tanh_t[:, ub, tloc * S : (tloc + 1) * S],
                                    start=(ub == 0), stop=(ub == 1),
                                    tile_position=(0, 32 * i),
                                )
                    scstg = perb.tile([128, 2, S], F32, tag="scstg")
                    nc.vector.scalar_tensor_tensor(
                        out=scstg,
                        in0=scq,
                        scalar=1.0,
                        in1=maskb,
                        op0=mybir.AluOpType.mult,
                        op1=mybir.AluOpType.add,
                    )
                    for h in range(2):
                        t0 = tg * TC + qr * 8 + h * 4
                        nc.gpsimd.dma_start(
                            out=scores_sb[t0 : t0 + 4, :],
                            in_=scstg[:, h, :].rearrange(
                                "(a r) s -> a r s", r=32
                            )[:, 0, :],
                        )

            prev = None
            for tg in range(T // TC):
                tanh_t = tanh_pool.tile([128, 2, TC * S], F16, tag="tanh")
                # rows via ScalarE fused tanh(k + q_bias): balances VectorE vs
                # ScalarE load; front-loaded (first group of the kernel fully
                # direct so ScalarE has work while VectorE fills the pipeline,
                # none at the end so ScalarE isn't the straggler)
                ndir = {
                    (0, 0): TG, (0, 1): 3, (0, 2): 3, (0, 3): 2,
                    (1, 0): 2, (1, 1): 1, (1, 2): 0, (1, 3): 0,
                }[(b, tg)]
                for i in range(ndir):
                    t = tg * TC + i
                    for ub in range(2):
                        nc.scalar.activation(
                            out=tanh_t[:, ub, i * S : (i + 1) * S],
                            in_=kT16[:, ub, :],
                            func=AF.Tanh,
                            bias=qT32[:, ub, t : t + 1],
                        )
                for grp in range(TC // TG):
                    g0 = grp * TG
                    lo = max(ndir - g0, 0)
                    if lo < TG:
                        # outer sums for TG t-rows, both u-blocks (DVE fp16)
                        ob = ob_pool.tile([128, 2, TG, S], F16, tag="ob")
                        for i in range(lo, TG):
                            t = tg * TC + g0 + i
                            for ub in range(2):
                                nc.vector.tensor_scalar_add(
                                    out=ob[:, ub, i, :],
                                    in0=kT16[:, ub, :],
                                    scalar1=qT32[:, ub, t : t + 1],
                                )
                        # tanh on ScalarE; the kernel's very last pass is
                        # split in two so its latency doesn't sit fully in
                        # the batch tail
                        last = b == B - 1 and tg == T // TC - 1 and grp == 1
                        bounds = (
                            [(lo, TG // 2), (TG // 2, TG)] if last
                            else [(lo, TG)]
                        )
                        for (i0, i1) in bounds:
                            dst = tanh_t[
                                :, :, (g0 + i0) * S : (g0 + i1) * S
                            ].rearrange("p a (i s) -> p a i s", s=S)
                            nc.scalar.activation(
                                out=dst, in_=ob[:, :, i0:i1, :], func=AF.Tanh
                            )
                    # contraction for the PREVIOUS tanh pass: PE contracts
                    # group g while ScalarE computes group g+1
                    if prev is not None:
                        emit_contraction(*prev)
                    prev = (tg, grp, tanh_t)
            emit_contraction(*prev)

            # ---------------- softmax over s
            nc.scalar.activation(
                out=exp_sb, in_=scores_sb, func=AF.Exp, accum_out=sums
            )
            nc.vector.reciprocal(out=inv, in_=sums)
            nc.vector.tensor_scalar_mul(out=attn_f, in0=exp_sb, scalar1=inv)
            nc.sync.dma_start(out=attn_out[b], in_=attn_f)

            # ---------------- context = attn @ value
            attnT = perb.tile([128, 2, T], F16, tag="attnT")
            for sblk in range(2):
                pt = p_pre.tile([128, 128], F32, tag="pre")
                nc.tensor.transpose(pt, exp_sb[:, sblk * 128 : (sblk + 1) * 128], id32)
                nc.vector.tensor_copy(out=attnT[:, sblk, :], in_=pt)
            ctxp = p_pre.tile([T, D], F32, tag="pre")
            for sblk in range(2):
                nc.tensor.matmul(
                    ctxp,
                    lhsT=attnT[:, sblk, :],
                    rhs=v16[:, sblk, :],
                    start=(sblk == 0), stop=(sblk == 1),
                )
            ctx_f = perb.tile([T, D], F32, tag="ctx_f")
            nc.vector.tensor_scalar_mul(out=ctx_f, in0=ctxp, scalar1=inv)
            nc.sync.dma_start(out=ctx_out[b], in_=ctx_f)

    nc.compile()
    return nc


_BUILT: bass.Bass | None = None


def _get_built() -> bass.Bass:
    global _BUILT
    if _BUILT is None:
        _BUILT = build_bass()
    return _BUILT


def make_in_maps(query, value, mask, W1, W2, scale):
    q = np.ascontiguousarray(np.asarray(query, dtype=np.float32))
    v = np.ascontiguousarray(np.asarray(value, dtype=np.float32))
    m = np.ascontiguousarray(np.asarray(mask).astype(np.uint8))
    w1 = np.ascontiguousarray(np.asarray(W1, dtype=np.float32))
    w2 = np.ascontiguousarray(np.asarray(W2, dtype=np.float32))
    sc = np.ascontiguousarray(np.asarray(scale, dtype=np.float32))
    in_maps = []
    for c in range(N_CORES):
        sl = slice(B * c, B * (c + 1))
        in_maps.append(
            {
                "query": np.ascontiguousarray(q[sl]),
                "value": np.ascontiguousarray(v[sl]),
                "mask": np.ascontiguousarray(m[sl]),
                "W1": w1,
                "W2": w2,
                "scale": sc,
            }
        )
    return in_maps


def run(query, value, mask, W1, W2, scale, trace=False, **trace_kwargs):
    nc = _get_built()
    in_maps = make_in_maps(query, value, mask, W1, W2, scale)
    res = run_bass_kernel_spmd(
        nc, in_maps, core_ids=list(range(N_CORES)), trace=trace, **trace_kwargs
    )
    context = np.concatenate([r["context"] for r in res.results], axis=0)
    attn = np.concatenate([r["attn"] for r in res.results], axis=0)
    return (context, attn), res


def kernel(query, value, mask, W1, W2, scale):
    (context, attn), _ = run(query, value, mask, W1, W2, scale, trace=False)
    return context, attn


if __name__ == "__main__":
    build_bass()
    print("build OK")
